# revision 13
# baseline (speedup 1.0000x reference)
"""CoordAtt Trainium2 Bass kernel.

Reference computation (per batch n, c=256, h=w=64, mip=8):
    xs   = x + residual                      (bilinear resize of residual at
                                              identical shape is the identity)
    y    = concat(mean_w(xs), mean_h(xs))    -> [c, h+w]
    y    = hswish(BN(w1 @ y + b1))           -> [mip, h+w]
    a_h  = sigmoid(w2 @ y[:, :h] + b2)       -> [c, h]
    a_w  = sigmoid(w3 @ y[:, h:] + b3)       -> [c, w]
    out  = 2*xs*a_h*a_w + 2*residual*(1 - a_h*a_w)
         = 2*(a_h*a_w*x + residual)          (algebraically identical)

Kernel strategy (8 cores, data-parallel over batch n: 2 batches/core):
  * conv-before-pool: pooling and the 1x1 conv are both linear, so compute
    y_conv = w1^T @ x + w1^T @ res on the TensorEngine (PSUM accumulation
    gives the x+res add for free), then pool the tiny (mip, h*w) result.
  * BN folds into one per-partition scale/bias activation op.
  * final elementwise tail is only 3 ops/element, split row-wise between
    VectorE (DVE) and GpSimd, all in-place in the input tiles.
"""

import numpy as np

import concourse.bacc as bacc
import concourse.mybir as mybir
from concourse.tile import TileContext
from concourse.bass_utils import run_bass_kernel_spmd

F32 = mybir.dt.float32
Alu = mybir.AluOpType
Act = mybir.ActivationFunctionType
AX = mybir.AxisListType

N_CORES = 8
N, C, H, W = 16, 256, 64, 64
NLOC = N // N_CORES           # batches per core
MIP = 8
EPS = 1e-5
HW = H * W                    # 4096 free columns per (batch, c-chunk)
NCHUNK = C // 128             # c-chunk count (2)
SEG = 4                       # conv psum segments per (batch): 4 x 1024 cols
SEGH = H // SEG               # h rows per segment (16)
SEGCOL = SEGH * W             # columns per segment (1024)
# rows of the final elementwise handled by DVE; the rest go to GpSimd
DVE_ROWS = 44
# "row": split each (batch, chunk) tile row-wise between DVE and GpSimd
# "chunk": DVE takes chunk 0, GpSimd takes chunk 1 (disjoint tiles)
FINAL_SPLIT = "chunk"


ALL_STAGES = frozenset({"conv", "pools", "mlp", "final_dve", "final_gp"})


def build_module(stages=ALL_STAGES):
    nc = bacc.Bacc("TRN2", target_bir_lowering=False)

    x_d = nc.dram_tensor("x", (NLOC, C, H, W), F32, kind="ExternalInput")
    r_d = nc.dram_tensor("residual", (NLOC, C, H, W), F32, kind="ExternalInput")
    w1_d = nc.dram_tensor("w1", (MIP, C), F32, kind="ExternalInput")
    b1_d = nc.dram_tensor("b1", (MIP,), F32, kind="ExternalInput")
    gamma_d = nc.dram_tensor("bn_gamma", (MIP,), F32, kind="ExternalInput")
    beta_d = nc.dram_tensor("bn_beta", (MIP,), F32, kind="ExternalInput")
    mean_d = nc.dram_tensor("bn_mean", (MIP,), F32, kind="ExternalInput")
    var_d = nc.dram_tensor("bn_var", (MIP,), F32, kind="ExternalInput")
    w2_d = nc.dram_tensor("w2", (C, MIP), F32, kind="ExternalInput")
    b2_d = nc.dram_tensor("b2", (C,), F32, kind="ExternalInput")
    w3_d = nc.dram_tensor("w3", (C, MIP), F32, kind="ExternalInput")
    b3_d = nc.dram_tensor("b3", (C,), F32, kind="ExternalInput")
    out_d = nc.dram_tensor("out", (NLOC, C, H, W), F32, kind="ExternalOutput")

    with TileContext(nc) as tc:
        with (
            tc.tile_pool(name="big", bufs=1) as big,
            tc.tile_pool(name="small", bufs=1) as small,
            tc.tile_pool(name="work", bufs=2) as work,
            tc.tile_pool(name="psum_y", bufs=2, space="PSUM") as psum_y_pool,
            tc.tile_pool(name="psum_a", bufs=2, space="PSUM") as psum_a_pool,
        ):
            # ---- replicated constants ----
            # w1 chunk-transposed: (c128, mip) per c-chunk
            w1t = []
            for k in range(NCHUNK):
                t = small.tile([128, MIP], F32, name=f"w1t{k}", tag=f"w1t{k}")
                nc.sync.dma_start(t[:], w1_d[:, k * 128:(k + 1) * 128].rearrange("m c -> c m"))
                w1t.append(t)
            # w2/w3 transposed: (mip, C)
            w2t = small.tile([MIP, C], F32, tag="w2t")
            nc.sync.dma_start(w2t[:], w2_d.rearrange("o m -> m o"))
            w3t = small.tile([MIP, C], F32, tag="w3t")
            nc.sync.dma_start(w3t[:], w3_d.rearrange("o m -> m o"))
            # b2/b3 per-partition: (128, chunk)
            b2t = small.tile([128, NCHUNK], F32, tag="b2t")
            nc.sync.dma_start(b2t[:], b2_d.rearrange("(k p) -> p k", p=128))
            b3t = small.tile([128, NCHUNK], F32, tag="b3t")
            nc.sync.dma_start(b3t[:], b3_d.rearrange("(k p) -> p k", p=128))
            # BN constants, (mip, 1) per-partition scalars
            bn_in = small.tile([MIP, 5], F32, tag="bn_in")
            for i, d in enumerate((var_d, gamma_d, beta_d, mean_d, b1_d)):
                nc.sync.dma_start(bn_in[:, i:i + 1], d[:].unsqueeze(1))
            var_c = bn_in[:, 0:1]
            gamma_c = bn_in[:, 1:2]
            beta_c = bn_in[:, 2:3]
            mean_c = bn_in[:, 3:4]
            b1_c = bn_in[:, 4:5]

            consts = small.tile([128, 2], F32, tag="consts")
            nc.vector.memset(consts[:, 0:1], EPS)
            nc.vector.memset(consts[:, 1:2], 3.0)

            bn_t = small.tile([MIP, 4], F32, tag="bn_t")
            sv = bn_t[:, 0:1]       # sqrt(var+eps)
            inv = bn_t[:, 1:2]      # gamma / sqrt(var+eps)
            scale_p = bn_t[:, 2:3]  # inv / W   (pool-sum -> mean fold)
            bias_p = bn_t[:, 3:4]   # (b1 - mean) * inv + beta
            nc.scalar.activation(sv, var_c, Act.Sqrt, bias=consts[:MIP, 0:1], scale=1.0)
            nc.vector.reciprocal(inv, sv)
            nc.vector.tensor_tensor(inv, inv, gamma_c, Alu.mult)
            nc.vector.tensor_scalar_mul(scale_p, inv, 1.0 / W)
            nc.vector.tensor_tensor(bias_p, b1_c, mean_c, Alu.subtract)
            nc.vector.scalar_tensor_tensor(bias_p, bias_p, inv, beta_c, Alu.mult, Alu.add)
            # note: scalar AP operand must be per-partition [p,1]; inv is (MIP,1)

            for b in range(NLOC):
                # ---- load inputs ----
                xt, rt = [], []
                for k in range(NCHUNK):
                    cs = slice(k * 128, (k + 1) * 128)
                    xk = big.tile([128, HW], F32, name=f"x_{b}_{k}", tag=f"x{b}{k}")
                    nc.sync.dma_start(xk[:], x_d[b, cs].rearrange("c h w -> c (h w)"))
                    rk = big.tile([128, HW], F32, name=f"r_{b}_{k}", tag=f"r{b}{k}")
                    nc.sync.dma_start(rk[:], r_d[b, cs].rearrange("c h w -> c (h w)"))
                    xt.append(xk)
                    rt.append(rk)

                # ---- conv (c -> mip) + implicit x+res via PSUM accumulation,
                #      then directional pool sums ----
                yh_sum = work.tile([MIP, H], F32, name=f"yh_{b}", tag="yh")
                ywp = work.tile([MIP, SEG * W], F32, name=f"ywp_{b}", tag="ywp")
                nc.vector.memset(yh_sum[:], 0.01)
                nc.vector.memset(ywp[:], 0.01)
                for s in range(SEG if "conv" in stages else 0):
                    ypsum = psum_y_pool.tile([MIP, SEGCOL], F32, name=f"yp_{b}_{s}", tag="yp")
                    for j in range(0, SEGCOL, 512):
                        srcs = [(k, t) for k in range(NCHUNK) for t in (xt[k], rt[k])]
                        for i, (k, src) in enumerate(srcs):
                            nc.tensor.matmul(
                                ypsum[:, j:j + 512],
                                w1t[k][:, :MIP],
                                src[:, s * SEGCOL + j: s * SEGCOL + j + 512],
                                start=(i == 0),
                                stop=(i == len(srcs) - 1),
                            )
                    if "pools" in stages:
                        # row sums (over w) for this segment's h rows
                        nc.vector.reduce_sum(
                            yh_sum[:, s * SEGH:(s + 1) * SEGH],
                            ypsum.rearrange("m (h w) -> m h w", h=SEGH),
                            axis=AX.X,
                        )
                        # partial column sums (over this segment's h rows)
                        nc.vector.reduce_sum(
                            ywp[:, s * W:(s + 1) * W],
                            ypsum.rearrange("m (h w) -> m w h", h=SEGH),
                            axis=AX.X,
                        )
                yw_sum = work.tile([MIP, W], F32, name=f"yw_{b}", tag="yw")
                nc.vector.tensor_tensor(ywp[:, 0:W], ywp[:, 0:W], ywp[:, W:2 * W], Alu.add)
                nc.vector.tensor_tensor(ywp[:, 2 * W:3 * W], ywp[:, 2 * W:3 * W], ywp[:, 3 * W:4 * W], Alu.add)
                nc.vector.tensor_tensor(yw_sum[:], ywp[:, 0:W], ywp[:, 2 * W:3 * W], Alu.add)

                # ---- BN (folded) + hswish on the two tiny pooled maps ----
                if "mlp" in stages:
                    vhw = work.tile([MIP, H + W], F32, name=f"vhw_{b}", tag="vhw")
                    uhw = work.tile([MIP, H + W], F32, name=f"uhw_{b}", tag="uhw")
                    ybn = work.tile([MIP, H + W], F32, name=f"ybn_{b}", tag="ybn")
                    nc.scalar.activation(ybn[:, :H], yh_sum[:], Act.Identity, bias=bias_p, scale=scale_p)
                    nc.scalar.activation(ybn[:, H:], yw_sum[:], Act.Identity, bias=bias_p, scale=scale_p)
                    # hswish(y) = y * min(relu(y+3), 6) / 6
                    nc.scalar.activation(uhw[:], ybn[:], Act.Relu, bias=consts[:MIP, 1:2], scale=1.0)
                    nc.vector.tensor_scalar_min(uhw[:], uhw[:], 6.0)
                    nc.vector.scalar_tensor_tensor(vhw[:], uhw[:], 1.0 / 6.0, ybn[:], Alu.mult, Alu.mult)
                    v_h = vhw[:, :H]
                    v_w = vhw[:, H:]

                # ---- attention vectors: a_h2 = 2*sigmoid(w2 @ v_h + b2),
                #      a_w = sigmoid(w3 @ v_w + b3) ----
                ah2, aw = [], []
                for k in range(NCHUNK):
                    cs = slice(k * 128, (k + 1) * 128)
                    aht = work.tile([128, H], F32, name=f"ah_{b}_{k}", tag=f"ah{k}")
                    awt = work.tile([128, W], F32, name=f"aw_{b}_{k}", tag=f"aw{k}")
                    if "mlp" in stages:
                        ahp = psum_a_pool.tile([128, H], F32, name=f"ahp_{b}_{k}", tag="ahp")
                        nc.tensor.matmul(ahp[:], w2t[:, cs], v_h, start=True, stop=True)
                        nc.scalar.activation(aht[:], ahp[:], Act.Sigmoid, bias=b2t[:, k:k + 1], scale=1.0)
                        nc.vector.tensor_scalar_mul(aht[:], aht[:], 2.0)
                        awp = psum_a_pool.tile([128, W], F32, name=f"awp_{b}_{k}", tag="awp")
                        nc.tensor.matmul(awp[:], w3t[:, cs], v_w, start=True, stop=True)
                        nc.scalar.activation(awt[:], awp[:], Act.Sigmoid, bias=b3t[:, k:k + 1], scale=1.0)
                    else:
                        nc.vector.memset(aht[:], 1.0)
                        nc.vector.memset(awt[:], 0.5)
                    ah2.append(aht)
                    aw.append(awt)

                # ---- final elementwise: out = (2*a_h*a_w)*x + 2*res ----
                # split rows h in [0, DVE_ROWS) -> DVE, rest -> GpSimd
                for k in range(NCHUNK):
                    xv = xt[k].rearrange("p (h w) -> p h w", h=H)
                    rv = rt[k].rearrange("p (h w) -> p h w", h=H)
                    if FINAL_SPLIT == "row":
                        plan = (("dve", 0, DVE_ROWS), ("gp", DVE_ROWS, H))
                    else:
                        plan = ((("dve", 0, H),) if k == 0 else (("gp", 0, H),))
                    for eng, h0, h1 in plan:
                        if f"final_{eng}" not in stages:
                            continue
                        nh = h1 - h0
                        xs_ = xv[:, h0:h1, :]
                        rs_ = rv[:, h0:h1, :]
                        awb = aw[k].unsqueeze(1).broadcast_to((128, nh, W))
                        ahb = ah2[k][:, h0:h1].unsqueeze(2).broadcast_to((128, nh, W))
                        if eng == "dve":
                            nc.vector.tensor_tensor(xs_, xs_, awb, Alu.mult)
                            nc.vector.tensor_tensor(xs_, xs_, ahb, Alu.mult)
                            nc.vector.scalar_tensor_tensor(rs_, rs_, 2.0, xs_, Alu.mult, Alu.add)
                        else:
                            nc.gpsimd.tensor_tensor(xs_, xs_, awb, Alu.mult)
                            nc.gpsimd.tensor_tensor(xs_, xs_, ahb, Alu.mult)
                            nc.gpsimd.tensor_scalar_mul(rs_, rs_, 2.0)
                            nc.gpsimd.tensor_tensor(rs_, rs_, xs_, Alu.add)

                    # result lives in rt[k]
                    cs = slice(k * 128, (k + 1) * 128)
                    nc.sync.dma_start(out_d[b, cs].rearrange("c h w -> c (h w)"), rt[k][:])

    nc.compile()
    return nc


_NC_CACHE = None


def _get_module():
    global _NC_CACHE
    if _NC_CACHE is None:
        _NC_CACHE = build_module()
    return _NC_CACHE


def make_in_maps(inputs):
    reps = {k: np.ascontiguousarray(v) for k, v in inputs.items()
            if k not in ("x", "residual")}
    in_maps = []
    for core in range(N_CORES):
        bs = slice(core * NLOC, (core + 1) * NLOC)
        m = {"x": np.ascontiguousarray(inputs["x"][bs]),
             "residual": np.ascontiguousarray(inputs["residual"][bs])}
        m.update(reps)
        in_maps.append(m)
    return in_maps


def run_spmd(nc, in_maps):
    res = run_bass_kernel_spmd(nc, in_maps, core_ids=list(range(N_CORES)))
    return np.concatenate([res.results[c]["out"] for c in range(N_CORES)], axis=0)


def kernel(**inputs):
    inputs = {k: np.asarray(v) for k, v in inputs.items()}
    nc = _get_module()
    return run_spmd(nc, make_in_maps(inputs))


# revision 14
# speedup vs baseline: 1.0366x; 1.0366x over previous
"""CoordAtt Trainium2 Bass kernel.

Reference computation (per batch n, c=256, h=w=64, mip=8):
    xs   = x + residual                      (bilinear resize of residual at
                                              identical shape is the identity)
    y    = concat(mean_w(xs), mean_h(xs))    -> [c, h+w]
    y    = hswish(BN(w1 @ y + b1))           -> [mip, h+w]
    a_h  = sigmoid(w2 @ y[:, :h] + b2)       -> [c, h]
    a_w  = sigmoid(w3 @ y[:, h:] + b3)       -> [c, w]
    out  = 2*xs*a_h*a_w + 2*residual*(1 - a_h*a_w)
         = 2*(a_h*a_w*x + residual)          (algebraically identical)

Kernel strategy (8 cores, data-parallel over batch n: 2 batches/core):
  * conv-before-pool: pooling and the 1x1 conv are both linear, so compute
    y_conv = w1^T @ x + w1^T @ res on the TensorEngine (PSUM accumulation
    gives the x+res add for free), then pool the tiny (mip, h*w) result.
    Conv inputs are cast to bf16 on the Scalar engine (4x faster PE).
  * BN folds into one per-partition scale/bias activation op.
  * final elementwise tail is only 3 ops/element, split across VectorE (DVE)
    and GpSimd on h-half tiles (separate SBUF tensors per engine --
    concurrent DVE+GpSimd in-place writes to one tensor hang the device),
    all in-place in the input tiles.
"""

import numpy as np

import concourse.bacc as bacc
import concourse.mybir as mybir
from concourse.tile import TileContext
from concourse.bass_utils import run_bass_kernel_spmd

F32 = mybir.dt.float32
BF16 = mybir.dt.bfloat16
Alu = mybir.AluOpType
Act = mybir.ActivationFunctionType
AX = mybir.AxisListType

N_CORES = 8
N, C, H, W = 16, 256, 64, 64
NLOC = N // N_CORES           # batches per core
MIP = 8
EPS = 1e-5
HW = H * W                    # 4096 free columns per (batch, c-chunk)
NCHUNK = C // 128             # c-chunk count (2)
NHALF = 2                     # h-half split of each chunk tile
HCOL = HW // NHALF            # 2048 columns per half tile
SEG = 2                       # conv psum segments per (batch): 2 x 2048 cols
SEGH = H // SEG               # h rows per segment (32)
SEGCOL = SEGH * W             # columns per segment (2048)

# final elementwise: which (chunk, half) units go to GpSimd (rest on DVE)
GP_UNITS = frozenset({(0, 1), (1, 1)})
GP_STT = True                 # use fused scalar_tensor_tensor on GpSimd

ALL_STAGES = frozenset({"conv", "pools", "mlp", "final_dve", "final_gp"})


def build_module(stages=ALL_STAGES):
    nc = bacc.Bacc("TRN2", target_bir_lowering=False)

    x_d = nc.dram_tensor("x", (NLOC, C, H, W), F32, kind="ExternalInput")
    r_d = nc.dram_tensor("residual", (NLOC, C, H, W), F32, kind="ExternalInput")
    w1_d = nc.dram_tensor("w1", (MIP, C), F32, kind="ExternalInput")
    b1_d = nc.dram_tensor("b1", (MIP,), F32, kind="ExternalInput")
    gamma_d = nc.dram_tensor("bn_gamma", (MIP,), F32, kind="ExternalInput")
    beta_d = nc.dram_tensor("bn_beta", (MIP,), F32, kind="ExternalInput")
    mean_d = nc.dram_tensor("bn_mean", (MIP,), F32, kind="ExternalInput")
    var_d = nc.dram_tensor("bn_var", (MIP,), F32, kind="ExternalInput")
    w2_d = nc.dram_tensor("w2", (C, MIP), F32, kind="ExternalInput")
    b2_d = nc.dram_tensor("b2", (C,), F32, kind="ExternalInput")
    w3_d = nc.dram_tensor("w3", (C, MIP), F32, kind="ExternalInput")
    b3_d = nc.dram_tensor("b3", (C,), F32, kind="ExternalInput")
    out_d = nc.dram_tensor("out", (NLOC, C, H, W), F32, kind="ExternalOutput")

    with TileContext(nc) as tc:
        with (
            tc.tile_pool(name="big", bufs=1) as big,
            tc.tile_pool(name="bfc", bufs=4) as bfc,
            tc.tile_pool(name="small", bufs=1) as small,
            tc.tile_pool(name="work", bufs=2) as work,
            tc.tile_pool(name="psum_y", bufs=1, space="PSUM") as psum_y_pool,
            tc.tile_pool(name="psum_a", bufs=2, space="PSUM") as psum_a_pool,
        ):
            # ---- replicated constants ----
            # w1 chunk-transposed: (c128, mip) per c-chunk, cast to bf16
            w1t = []
            for k in range(NCHUNK):
                tf = small.tile([128, MIP], F32, name=f"w1tf{k}", tag=f"w1tf{k}")
                nc.sync.dma_start(tf[:], w1_d[:, k * 128:(k + 1) * 128].rearrange("m c -> c m"))
                tb = small.tile([128, MIP], BF16, name=f"w1t{k}", tag=f"w1t{k}")
                nc.scalar.copy(tb[:], tf[:])
                w1t.append(tb)
            # w2/w3 transposed: (mip, C)
            w2t = small.tile([MIP, C], F32, tag="w2t")
            nc.sync.dma_start(w2t[:], w2_d.rearrange("o m -> m o"))
            w3t = small.tile([MIP, C], F32, tag="w3t")
            nc.sync.dma_start(w3t[:], w3_d.rearrange("o m -> m o"))
            # b2/b3 per-partition: (128, chunk)
            b2t = small.tile([128, NCHUNK], F32, tag="b2t")
            nc.sync.dma_start(b2t[:], b2_d.rearrange("(k p) -> p k", p=128))
            b3t = small.tile([128, NCHUNK], F32, tag="b3t")
            nc.sync.dma_start(b3t[:], b3_d.rearrange("(k p) -> p k", p=128))
            # BN constants, (mip, 1) per-partition scalars
            bn_in = small.tile([MIP, 5], F32, tag="bn_in")
            for i, d in enumerate((var_d, gamma_d, beta_d, mean_d, b1_d)):
                nc.sync.dma_start(bn_in[:, i:i + 1], d[:].unsqueeze(1))
            var_c = bn_in[:, 0:1]
            gamma_c = bn_in[:, 1:2]
            beta_c = bn_in[:, 2:3]
            mean_c = bn_in[:, 3:4]
            b1_c = bn_in[:, 4:5]

            consts = small.tile([128, 2], F32, tag="consts")
            nc.vector.memset(consts[:, 0:1], EPS)
            nc.vector.memset(consts[:, 1:2], 3.0)

            bn_t = small.tile([MIP, 4], F32, tag="bn_t")
            sv = bn_t[:, 0:1]       # sqrt(var+eps)
            inv = bn_t[:, 1:2]      # gamma / sqrt(var+eps)
            scale_p = bn_t[:, 2:3]  # inv / W   (pool-sum -> mean fold)
            bias_p = bn_t[:, 3:4]   # (b1 - mean) * inv + beta
            nc.scalar.activation(sv, var_c, Act.Sqrt, bias=consts[:MIP, 0:1], scale=1.0)
            nc.vector.reciprocal(inv, sv)
            nc.vector.tensor_tensor(inv, inv, gamma_c, Alu.mult)
            nc.vector.tensor_scalar_mul(scale_p, inv, 1.0 / W)
            nc.vector.tensor_tensor(bias_p, b1_c, mean_c, Alu.subtract)
            nc.vector.scalar_tensor_tensor(bias_p, bias_p, inv, beta_c, Alu.mult, Alu.add)

            for b in range(NLOC):
                # ---- load inputs as (b, chunk, half) tiles; cast bf16 copies ----
                xt = {}
                rt = {}
                xb = {}
                rb = {}
                for k in range(NCHUNK):
                    cs = slice(k * 128, (k + 1) * 128)
                    xd = x_d[b, cs].rearrange("c h w -> c (h w)")
                    rd = r_d[b, cs].rearrange("c h w -> c (h w)")
                    for j in range(NHALF):
                        js = slice(j * HCOL, (j + 1) * HCOL)
                        t = big.tile([128, HCOL], F32, name=f"x_{b}_{k}_{j}", tag=f"x{b}{k}{j}")
                        nc.sync.dma_start(t[:], xd[:, js])
                        xt[k, j] = t
                        t = big.tile([128, HCOL], F32, name=f"r_{b}_{k}_{j}", tag=f"r{b}{k}{j}")
                        nc.sync.dma_start(t[:], rd[:, js])
                        rt[k, j] = t
                        tb = bfc.tile([128, HCOL], BF16, name=f"xb_{b}_{k}_{j}", tag="xb")
                        nc.scalar.copy(tb[:], xt[k, j][:])
                        xb[k, j] = tb
                        tb = bfc.tile([128, HCOL], BF16, name=f"rb_{b}_{k}_{j}", tag="rb")
                        nc.scalar.copy(tb[:], rt[k, j][:])
                        rb[k, j] = tb

                # ---- conv (c -> mip) + implicit x+res via PSUM accumulation,
                #      then directional pool sums ----
                yh_sum = work.tile([MIP, H], F32, name=f"yh_{b}", tag="yh")
                ywp = work.tile([MIP, SEG * W], F32, name=f"ywp_{b}", tag="ywp")
                nc.vector.memset(yh_sum[:], 0.01)
                nc.vector.memset(ywp[:], 0.01)
                for s in range(SEG if "conv" in stages else 0):
                    # segment s covers h rows [s*SEGH, (s+1)*SEGH) = half tile s
                    ypsum = psum_y_pool.tile([MIP, SEGCOL], F32, name=f"yp_{b}_{s}", tag="yp")
                    for jj in range(0, SEGCOL, 512):
                        srcs = [(k, t) for k in range(NCHUNK)
                                for t in (xb[k, s], rb[k, s])]
                        for i, (k, src) in enumerate(srcs):
                            nc.tensor.matmul(
                                ypsum[:, jj:jj + 512],
                                w1t[k][:, :MIP],
                                src[:, jj:jj + 512],
                                start=(i == 0),
                                stop=(i == len(srcs) - 1),
                            )
                    if "pools" in stages:
                        # row sums (over w) for this segment's h rows
                        nc.vector.reduce_sum(
                            yh_sum[:, s * SEGH:(s + 1) * SEGH],
                            ypsum.rearrange("m (h w) -> m h w", h=SEGH),
                            axis=AX.X,
                        )
                        # partial column sums (over this segment's h rows)
                        nc.vector.reduce_sum(
                            ywp[:, s * W:(s + 1) * W],
                            ypsum.rearrange("m (h w) -> m w h", h=SEGH),
                            axis=AX.X,
                        )
                yw_sum = work.tile([MIP, W], F32, name=f"yw_{b}", tag="yw")
                nc.vector.tensor_tensor(yw_sum[:], ywp[:, 0:W], ywp[:, W:2 * W], Alu.add)

                # ---- BN (folded) + hswish on the two tiny pooled maps ----
                if "mlp" in stages:
                    vhw = work.tile([MIP, H + W], F32, name=f"vhw_{b}", tag="vhw")
                    uhw = work.tile([MIP, H + W], F32, name=f"uhw_{b}", tag="uhw")
                    ybn = work.tile([MIP, H + W], F32, name=f"ybn_{b}", tag="ybn")
                    nc.scalar.activation(ybn[:, :H], yh_sum[:], Act.Identity, bias=bias_p, scale=scale_p)
                    nc.scalar.activation(ybn[:, H:], yw_sum[:], Act.Identity, bias=bias_p, scale=scale_p)
                    # hswish(y) = y * min(relu(y+3), 6) / 6
                    nc.scalar.activation(uhw[:], ybn[:], Act.Relu, bias=consts[:MIP, 1:2], scale=1.0)
                    nc.vector.tensor_scalar_min(uhw[:], uhw[:], 6.0)
                    nc.vector.scalar_tensor_tensor(vhw[:], uhw[:], 1.0 / 6.0, ybn[:], Alu.mult, Alu.mult)
                    v_h = vhw[:, :H]
                    v_w = vhw[:, H:]

                # ---- attention vectors: a_h2 = 2*sigmoid(w2 @ v_h + b2),
                #      a_w = sigmoid(w3 @ v_w + b3) ----
                ah2, aw = [], []
                for k in range(NCHUNK):
                    cs = slice(k * 128, (k + 1) * 128)
                    aht = work.tile([128, H], F32, name=f"ah_{b}_{k}", tag=f"ah{k}")
                    awt = work.tile([128, W], F32, name=f"aw_{b}_{k}", tag=f"aw{k}")
                    if "mlp" in stages:
                        ahp = psum_a_pool.tile([128, H], F32, name=f"ahp_{b}_{k}", tag="ahp")
                        nc.tensor.matmul(ahp[:], w2t[:, cs], v_h, start=True, stop=True)
                        nc.scalar.activation(aht[:], ahp[:], Act.Sigmoid, bias=b2t[:, k:k + 1], scale=1.0)
                        nc.vector.tensor_scalar_mul(aht[:], aht[:], 2.0)
                        awp = psum_a_pool.tile([128, W], F32, name=f"awp_{b}_{k}", tag="awp")
                        nc.tensor.matmul(awp[:], w3t[:, cs], v_w, start=True, stop=True)
                        nc.scalar.activation(awt[:], awp[:], Act.Sigmoid, bias=b3t[:, k:k + 1], scale=1.0)
                    else:
                        nc.vector.memset(aht[:], 1.0)
                        nc.vector.memset(awt[:], 0.5)
                    ah2.append(aht)
                    aw.append(awt)

                # ---- final elementwise: out = (2*a_h*a_w)*x + 2*res ----
                for k in range(NCHUNK):
                    cs = slice(k * 128, (k + 1) * 128)
                    od = out_d[b, cs].rearrange("c h w -> c (h w)")
                    for j in range(NHALF):
                        h0 = j * (H // NHALF)
                        h1 = (j + 1) * (H // NHALF)
                        nh = h1 - h0
                        eng = "gp" if (k, j) in GP_UNITS else "dve"
                        if f"final_{eng}" not in stages:
                            continue
                        xs_ = xt[k, j].rearrange("p (h w) -> p h w", h=nh)
                        rs_ = rt[k, j].rearrange("p (h w) -> p h w", h=nh)
                        awb = aw[k].unsqueeze(1).broadcast_to((128, nh, W))
                        ahb = ah2[k][:, h0:h1].unsqueeze(2).broadcast_to((128, nh, W))
                        if eng == "dve":
                            nc.vector.tensor_tensor(xs_, xs_, awb, Alu.mult)
                            nc.vector.tensor_tensor(xs_, xs_, ahb, Alu.mult)
                            nc.vector.scalar_tensor_tensor(rs_, rs_, 2.0, xs_, Alu.mult, Alu.add)
                        else:
                            nc.gpsimd.tensor_tensor(xs_, xs_, awb, Alu.mult)
                            nc.gpsimd.tensor_tensor(xs_, xs_, ahb, Alu.mult)
                            if GP_STT:
                                nc.gpsimd.scalar_tensor_tensor(rs_, rs_, 2.0, xs_, Alu.mult, Alu.add)
                            else:
                                nc.gpsimd.tensor_scalar_mul(rs_, rs_, 2.0)
                                nc.gpsimd.tensor_tensor(rs_, rs_, xs_, Alu.add)

                        # result lives in rt[k, j]
                        nc.sync.dma_start(od[:, j * HCOL:(j + 1) * HCOL], rt[k, j][:])

    nc.compile()
    return nc


_NC_CACHE = None


def _get_module():
    global _NC_CACHE
    if _NC_CACHE is None:
        _NC_CACHE = build_module()
    return _NC_CACHE


def make_in_maps(inputs):
    reps = {k: np.ascontiguousarray(v) for k, v in inputs.items()
            if k not in ("x", "residual")}
    in_maps = []
    for core in range(N_CORES):
        bs = slice(core * NLOC, (core + 1) * NLOC)
        m = {"x": np.ascontiguousarray(inputs["x"][bs]),
             "residual": np.ascontiguousarray(inputs["residual"][bs])}
        m.update(reps)
        in_maps.append(m)
    return in_maps


def run_spmd(nc, in_maps):
    res = run_bass_kernel_spmd(nc, in_maps, core_ids=list(range(N_CORES)))
    return np.concatenate([res.results[c]["out"] for c in range(N_CORES)], axis=0)


def kernel(**inputs):
    inputs = {k: np.asarray(v) for k, v in inputs.items()}
    nc = _get_module()
    return run_spmd(nc, make_in_maps(inputs))


# revision 15
# speedup vs baseline: 1.2797x; 1.2345x over previous
"""CoordAtt Trainium2 Bass kernel.

Reference computation (per batch n, c=256, h=w=64, mip=8):
    xs   = x + residual                      (bilinear resize of residual at
                                              identical shape is the identity)
    y    = concat(mean_w(xs), mean_h(xs))    -> [c, h+w]
    y    = hswish(BN(w1 @ y + b1))           -> [mip, h+w]
    a_h  = sigmoid(w2 @ y[:, :h] + b2)       -> [c, h]
    a_w  = sigmoid(w3 @ y[:, h:] + b3)       -> [c, w]
    out  = 2*xs*a_h*a_w + 2*residual*(1 - a_h*a_w)
         = 2*(a_h*a_w*x + residual)          (algebraically identical)

Kernel strategy (8 cores, data-parallel over batch n: 2 batches/core):
  * conv-before-pool: pooling and the 1x1 conv are both linear, so compute
    y_conv = w1^T @ x + w1^T @ res on the TensorEngine (PSUM accumulation
    gives the x+res add for free), then pool the tiny (mip, h*w) result.
    Matmuls run in float32r mode: fp32 data at full PE rate.
  * BN folds into one per-partition scale/bias activation op.
  * final elementwise tail is only 3 ops/element, split across VectorE (DVE)
    and GpSimd on h-half tiles (separate SBUF tensors per engine --
    concurrent DVE+GpSimd in-place writes to one tensor hang the device),
    all in-place in the input tiles.
  * two emission phases (all pool/attention work for every batch first, then
    all finals) so batch i+1's pipeline overlaps batch i's elementwise tail.
"""

import numpy as np

import concourse.bacc as bacc
import concourse.mybir as mybir
from concourse.tile import TileContext
from concourse.bass_utils import run_bass_kernel_spmd

F32 = mybir.dt.float32
F32R = mybir.dt.float32r
Alu = mybir.AluOpType
Act = mybir.ActivationFunctionType
AX = mybir.AxisListType

N_CORES = 8
N, C, H, W = 16, 256, 64, 64
NLOC = N // N_CORES           # batches per core
MIP = 8
EPS = 1e-5
HW = H * W                    # 4096 free columns per (batch, c-chunk)
NCHUNK = C // 128             # c-chunk count (2)
NHALF = 2                     # h-half split of each chunk tile
HCOL = HW // NHALF            # 2048 columns per half tile
SEG = 2                       # conv psum segments per batch: 2 x 2048 cols
SEGH = H // SEG               # h rows per segment (32)
SEGCOL = SEGH * W             # columns per segment (2048)

# final elementwise: which (chunk, half) units go to GpSimd (rest on DVE)
GP_UNITS = frozenset({(0, 1), (1, 1)})
GP_STT = True                 # use fused scalar_tensor_tensor on GpSimd

ALL_STAGES = frozenset({"conv", "pools", "mlp", "final_dve", "final_gp"})


def build_module(stages=ALL_STAGES):
    nc = bacc.Bacc("TRN2", target_bir_lowering=False)

    x_d = nc.dram_tensor("x", (NLOC, C, H, W), F32, kind="ExternalInput")
    r_d = nc.dram_tensor("residual", (NLOC, C, H, W), F32, kind="ExternalInput")
    w1_d = nc.dram_tensor("w1", (MIP, C), F32, kind="ExternalInput")
    b1_d = nc.dram_tensor("b1", (MIP,), F32, kind="ExternalInput")
    gamma_d = nc.dram_tensor("bn_gamma", (MIP,), F32, kind="ExternalInput")
    beta_d = nc.dram_tensor("bn_beta", (MIP,), F32, kind="ExternalInput")
    mean_d = nc.dram_tensor("bn_mean", (MIP,), F32, kind="ExternalInput")
    var_d = nc.dram_tensor("bn_var", (MIP,), F32, kind="ExternalInput")
    w2_d = nc.dram_tensor("w2", (C, MIP), F32, kind="ExternalInput")
    b2_d = nc.dram_tensor("b2", (C,), F32, kind="ExternalInput")
    w3_d = nc.dram_tensor("w3", (C, MIP), F32, kind="ExternalInput")
    b3_d = nc.dram_tensor("b3", (C,), F32, kind="ExternalInput")
    out_d = nc.dram_tensor("out", (NLOC, C, H, W), F32, kind="ExternalOutput")

    with TileContext(nc) as tc:
        with (
            tc.tile_pool(name="big", bufs=1) as big,
            tc.tile_pool(name="small", bufs=1) as small,
            tc.tile_pool(name="work", bufs=2) as work,
            tc.tile_pool(name="psum_y", bufs=1, space="PSUM") as psum_y_pool,
            tc.tile_pool(name="psum_a", bufs=2, space="PSUM") as psum_a_pool,
        ):
            # ---- replicated constants ----
            # w1 chunk-transposed: (c128, mip) per c-chunk
            w1t = []
            for k in range(NCHUNK):
                t = small.tile([128, MIP], F32, name=f"w1t{k}", tag=f"w1t{k}")
                nc.sync.dma_start(t[:], w1_d[:, k * 128:(k + 1) * 128].rearrange("m c -> c m"))
                w1t.append(t)
            # w2/w3 transposed: (mip, C)
            w2t = small.tile([MIP, C], F32, tag="w2t")
            nc.sync.dma_start(w2t[:], w2_d.rearrange("o m -> m o"))
            w3t = small.tile([MIP, C], F32, tag="w3t")
            nc.sync.dma_start(w3t[:], w3_d.rearrange("o m -> m o"))
            # b2/b3 per-partition: (128, chunk)
            b2t = small.tile([128, NCHUNK], F32, tag="b2t")
            nc.sync.dma_start(b2t[:], b2_d.rearrange("(k p) -> p k", p=128))
            b3t = small.tile([128, NCHUNK], F32, tag="b3t")
            nc.sync.dma_start(b3t[:], b3_d.rearrange("(k p) -> p k", p=128))
            # BN constants, (mip, 1) per-partition scalars
            bn_in = small.tile([MIP, 5], F32, tag="bn_in")
            for i, d in enumerate((var_d, gamma_d, beta_d, mean_d, b1_d)):
                nc.sync.dma_start(bn_in[:, i:i + 1], d[:].unsqueeze(1))
            var_c = bn_in[:, 0:1]
            gamma_c = bn_in[:, 1:2]
            beta_c = bn_in[:, 2:3]
            mean_c = bn_in[:, 3:4]
            b1_c = bn_in[:, 4:5]

            consts = small.tile([128, 2], F32, tag="consts")
            nc.vector.memset(consts[:, 0:1], EPS)
            nc.vector.memset(consts[:, 1:2], 3.0)

            bn_t = small.tile([MIP, 4], F32, tag="bn_t")
            sv = bn_t[:, 0:1]       # sqrt(var+eps)
            inv = bn_t[:, 1:2]      # gamma / sqrt(var+eps)
            scale_p = bn_t[:, 2:3]  # inv / W   (pool-sum -> mean fold)
            bias_p = bn_t[:, 3:4]   # (b1 - mean) * inv + beta
            nc.scalar.activation(sv, var_c, Act.Sqrt, bias=consts[:MIP, 0:1], scale=1.0)
            nc.vector.reciprocal(inv, sv)
            nc.vector.tensor_tensor(inv, inv, gamma_c, Alu.mult)
            nc.vector.tensor_scalar_mul(scale_p, inv, 1.0 / W)
            nc.vector.tensor_tensor(bias_p, b1_c, mean_c, Alu.subtract)
            nc.vector.scalar_tensor_tensor(bias_p, bias_p, inv, beta_c, Alu.mult, Alu.add)

            xt = {}
            rt = {}
            ah2 = {}
            aw = {}

            # ---- phase 1 (per batch): load, conv, pools, attention ----
            for b in range(NLOC):
                for j in range(NHALF):
                    js = slice(j * HCOL, (j + 1) * HCOL)
                    for k in range(NCHUNK):
                        cs = slice(k * 128, (k + 1) * 128)
                        t = big.tile([128, HCOL], F32, name=f"x_{b}_{k}_{j}", tag=f"x{b}{k}{j}")
                        nc.sync.dma_start(t[:], x_d[b, cs].rearrange("c h w -> c (h w)")[:, js])
                        xt[b, k, j] = t
                        t = big.tile([128, HCOL], F32, name=f"r_{b}_{k}_{j}", tag=f"r{b}{k}{j}")
                        nc.sync.dma_start(t[:], r_d[b, cs].rearrange("c h w -> c (h w)")[:, js])
                        rt[b, k, j] = t

                # conv (c -> mip) + implicit x+res via PSUM accumulation,
                # then directional pool sums
                yh_sum = work.tile([MIP, H], F32, name=f"yh_{b}", tag="yh")
                ywp = work.tile([MIP, SEG * W], F32, name=f"ywp_{b}", tag="ywp")
                if "conv" not in stages or "pools" not in stages:
                    nc.vector.memset(yh_sum[:], 0.01)
                    nc.vector.memset(ywp[:], 0.01)
                for s in range(SEG if "conv" in stages else 0):
                    # segment s covers h rows [s*SEGH, (s+1)*SEGH) = half tile s
                    ypsum = psum_y_pool.tile([MIP, SEGCOL], F32, name=f"yp_{b}_{s}", tag="yp")
                    for jj in range(0, SEGCOL, 512):
                        srcs = [(k, t) for k in range(NCHUNK)
                                for t in (xt[b, k, s], rt[b, k, s])]
                        for i, (k, src) in enumerate(srcs):
                            nc.tensor.matmul(
                                ypsum[:, jj:jj + 512],
                                w1t[k][:, :MIP].bitcast(F32R),
                                src[:, jj:jj + 512].bitcast(F32R),
                                start=(i == 0),
                                stop=(i == len(srcs) - 1),
                            )
                    if "pools" in stages:
                        # row sums (over w) for this segment's h rows
                        nc.vector.reduce_sum(
                            yh_sum[:, s * SEGH:(s + 1) * SEGH],
                            ypsum.rearrange("m (h w) -> m h w", h=SEGH),
                            axis=AX.X,
                        )
                        # partial column sums (over this segment's h rows)
                        nc.vector.reduce_sum(
                            ywp[:, s * W:(s + 1) * W],
                            ypsum.rearrange("m (h w) -> m w h", h=SEGH),
                            axis=AX.X,
                        )
                yw_sum = work.tile([MIP, W], F32, name=f"yw_{b}", tag="yw")
                nc.vector.tensor_tensor(yw_sum[:], ywp[:, 0:W], ywp[:, W:2 * W], Alu.add)

                # BN (folded) + hswish on the two tiny pooled maps
                if "mlp" in stages:
                    vhw = work.tile([MIP, H + W], F32, name=f"vhw_{b}", tag="vhw")
                    uhw = work.tile([MIP, H + W], F32, name=f"uhw_{b}", tag="uhw")
                    ybn = work.tile([MIP, H + W], F32, name=f"ybn_{b}", tag="ybn")
                    nc.scalar.activation(ybn[:, :H], yh_sum[:], Act.Identity, bias=bias_p, scale=scale_p)
                    nc.scalar.activation(ybn[:, H:], yw_sum[:], Act.Identity, bias=bias_p, scale=scale_p)
                    # hswish(y) = y * min(relu(y+3), 6) / 6
                    nc.scalar.activation(uhw[:], ybn[:], Act.Relu, bias=consts[:MIP, 1:2], scale=1.0)
                    nc.vector.tensor_scalar_min(uhw[:], uhw[:], 6.0)
                    nc.vector.scalar_tensor_tensor(vhw[:], uhw[:], 1.0 / 6.0, ybn[:], Alu.mult, Alu.mult)
                    v_h = vhw[:, :H]
                    v_w = vhw[:, H:]

                # attention vectors: a_h2 = 2*sigmoid(w2 @ v_h + b2),
                # a_w = sigmoid(w3 @ v_w + b3)
                for k in range(NCHUNK):
                    cs = slice(k * 128, (k + 1) * 128)
                    aht = work.tile([128, H], F32, name=f"ah_{b}_{k}", tag=f"ah{k}")
                    awt = work.tile([128, W], F32, name=f"aw_{b}_{k}", tag=f"aw{k}")
                    if "mlp" in stages:
                        ahp = psum_a_pool.tile([128, H], F32, name=f"ahp_{b}_{k}", tag="ahp")
                        nc.tensor.matmul(ahp[:], w2t[:, cs], v_h, start=True, stop=True)
                        # fold the final x2 into a_h via activation scale on copy
                        nc.scalar.activation(aht[:], ahp[:], Act.Sigmoid, bias=b2t[:, k:k + 1], scale=1.0)
                        nc.vector.tensor_scalar_mul(aht[:], aht[:], 2.0)
                        awp = psum_a_pool.tile([128, W], F32, name=f"awp_{b}_{k}", tag="awp")
                        nc.tensor.matmul(awp[:], w3t[:, cs], v_w, start=True, stop=True)
                        nc.scalar.activation(awt[:], awp[:], Act.Sigmoid, bias=b3t[:, k:k + 1], scale=1.0)
                    else:
                        nc.vector.memset(aht[:], 1.0)
                        nc.vector.memset(awt[:], 0.5)
                    ah2[b, k] = aht
                    aw[b, k] = awt

            # ---- phase 2 (per batch): final elementwise + store ----
            # out = (2*a_h*a_w)*x + 2*res
            for b in range(NLOC):
                for k in range(NCHUNK):
                    cs = slice(k * 128, (k + 1) * 128)
                    od = out_d[b, cs].rearrange("c h w -> c (h w)")
                    for j in range(NHALF):
                        h0 = j * (H // NHALF)
                        h1 = (j + 1) * (H // NHALF)
                        nh = h1 - h0
                        eng = "gp" if (k, j) in GP_UNITS else "dve"
                        if f"final_{eng}" not in stages:
                            continue
                        xs_ = xt[b, k, j].rearrange("p (h w) -> p h w", h=nh)
                        rs_ = rt[b, k, j].rearrange("p (h w) -> p h w", h=nh)
                        awb = aw[b, k].unsqueeze(1).broadcast_to((128, nh, W))
                        ahb = ah2[b, k][:, h0:h1].unsqueeze(2).broadcast_to((128, nh, W))
                        if eng == "dve":
                            nc.vector.tensor_tensor(xs_, xs_, awb, Alu.mult)
                            nc.vector.tensor_tensor(xs_, xs_, ahb, Alu.mult)
                            nc.vector.scalar_tensor_tensor(rs_, rs_, 2.0, xs_, Alu.mult, Alu.add)
                        elif GP_STT:
                            nc.gpsimd.scalar_tensor_tensor(xs_, xs_, 1.0, awb, Alu.mult, Alu.mult)
                            nc.gpsimd.scalar_tensor_tensor(xs_, xs_, 1.0, ahb, Alu.mult, Alu.mult)
                            nc.gpsimd.scalar_tensor_tensor(rs_, rs_, 2.0, xs_, Alu.mult, Alu.add)
                        else:
                            nc.gpsimd.tensor_tensor(xs_, xs_, awb, Alu.mult)
                            nc.gpsimd.tensor_tensor(xs_, xs_, ahb, Alu.mult)
                            nc.gpsimd.tensor_scalar_mul(rs_, rs_, 2.0)
                            nc.gpsimd.tensor_tensor(rs_, rs_, xs_, Alu.add)

                        # result lives in rt[b, k, j]
                        nc.sync.dma_start(od[:, j * HCOL:(j + 1) * HCOL], rt[b, k, j][:])

    nc.compile()
    return nc


_NC_CACHE = None


def _get_module():
    global _NC_CACHE
    if _NC_CACHE is None:
        _NC_CACHE = build_module()
    return _NC_CACHE


def make_in_maps(inputs):
    reps = {k: np.ascontiguousarray(v) for k, v in inputs.items()
            if k not in ("x", "residual")}
    in_maps = []
    for core in range(N_CORES):
        bs = slice(core * NLOC, (core + 1) * NLOC)
        m = {"x": np.ascontiguousarray(inputs["x"][bs]),
             "residual": np.ascontiguousarray(inputs["residual"][bs])}
        m.update(reps)
        in_maps.append(m)
    return in_maps


def run_spmd(nc, in_maps):
    res = run_bass_kernel_spmd(nc, in_maps, core_ids=list(range(N_CORES)))
    return np.concatenate([res.results[c]["out"] for c in range(N_CORES)], axis=0)


def kernel(**inputs):
    inputs = {k: np.asarray(v) for k, v in inputs.items()}
    nc = _get_module()
    return run_spmd(nc, make_in_maps(inputs))


# revision 16
# speedup vs baseline: 1.3301x; 1.0394x over previous
"""CoordAtt Trainium2 Bass kernel.

Reference computation (per batch n, c=256, h=w=64, mip=8):
    xs   = x + residual                      (bilinear resize of residual at
                                              identical shape is the identity)
    y    = concat(mean_w(xs), mean_h(xs))    -> [c, h+w]
    y    = hswish(BN(w1 @ y + b1))           -> [mip, h+w]
    a_h  = sigmoid(w2 @ y[:, :h] + b2)       -> [c, h]
    a_w  = sigmoid(w3 @ y[:, h:] + b3)       -> [c, w]
    out  = 2*xs*a_h*a_w + 2*residual*(1 - a_h*a_w)
         = 2*(a_h*a_w*x + residual)          (algebraically identical)

Kernel strategy (8 cores, data-parallel over batch n: 2 batches/core):
  * conv-before-pool: pooling and the 1x1 conv are both linear, so compute
    y_conv = w1^T @ x + w1^T @ res on the TensorEngine (PSUM accumulation
    gives the x+res add for free), then pool the tiny (mip, h*w) result.
    Matmuls run in float32r mode: fp32 data at full PE rate.
  * BN folds into one per-partition scale/bias activation op.
  * final elementwise tail is only 3 ops/element, split across VectorE (DVE)
    and GpSimd on h-half tiles (separate SBUF tensors per engine --
    concurrent DVE+GpSimd in-place writes to one tensor hang the device),
    all in-place in the input tiles.
  * two emission phases (all pool/attention work for every batch first, then
    all finals) so batch i+1's pipeline overlaps batch i's elementwise tail.
"""

import numpy as np

import concourse.bacc as bacc
import concourse.mybir as mybir
from concourse.tile import TileContext
from concourse.bass_utils import run_bass_kernel_spmd

F32 = mybir.dt.float32
F32R = mybir.dt.float32r
Alu = mybir.AluOpType
Act = mybir.ActivationFunctionType
AX = mybir.AxisListType

N_CORES = 8
N, C, H, W = 16, 256, 64, 64
NLOC = N // N_CORES           # batches per core
MIP = 8
EPS = 1e-5
HW = H * W                    # 4096 free columns per (batch, c-chunk)
NCHUNK = C // 128             # c-chunk count (2)
NHALF = 2                     # h-half split of each chunk tile
HCOL = HW // NHALF            # 2048 columns per half tile
SEG = 2                       # conv psum segments per batch: 2 x 2048 cols
SEGH = H // SEG               # h rows per segment (32)
SEGCOL = SEGH * W             # columns per segment (2048)

# final elementwise: which (chunk, half) units go to GpSimd (rest on DVE)
GP_UNITS = frozenset({(0, 1), (1, 1)})
GP_STT = True                 # use fused scalar_tensor_tensor on GpSimd

ALL_STAGES = frozenset({"conv", "pools", "mlp", "final_dve", "final_gp"})


def build_module(stages=ALL_STAGES):
    nc = bacc.Bacc("TRN2", target_bir_lowering=False)

    x_d = nc.dram_tensor("x", (NLOC, C, H, W), F32, kind="ExternalInput")
    r_d = nc.dram_tensor("residual", (NLOC, C, H, W), F32, kind="ExternalInput")
    w1_d = nc.dram_tensor("w1", (MIP, C), F32, kind="ExternalInput")
    b1_d = nc.dram_tensor("b1", (MIP,), F32, kind="ExternalInput")
    gamma_d = nc.dram_tensor("bn_gamma", (MIP,), F32, kind="ExternalInput")
    beta_d = nc.dram_tensor("bn_beta", (MIP,), F32, kind="ExternalInput")
    mean_d = nc.dram_tensor("bn_mean", (MIP,), F32, kind="ExternalInput")
    var_d = nc.dram_tensor("bn_var", (MIP,), F32, kind="ExternalInput")
    w2_d = nc.dram_tensor("w2", (C, MIP), F32, kind="ExternalInput")
    b2_d = nc.dram_tensor("b2", (C,), F32, kind="ExternalInput")
    w3_d = nc.dram_tensor("w3", (C, MIP), F32, kind="ExternalInput")
    b3_d = nc.dram_tensor("b3", (C,), F32, kind="ExternalInput")
    out_d = nc.dram_tensor("out", (NLOC, C, H, W), F32, kind="ExternalOutput")

    with TileContext(nc) as tc:
        with (
            tc.tile_pool(name="big", bufs=1) as big,
            tc.tile_pool(name="small", bufs=1) as small,
            tc.tile_pool(name="work", bufs=2) as work,
            tc.tile_pool(name="psum_y", bufs=1, space="PSUM") as psum_y_pool,
            tc.tile_pool(name="psum_a", bufs=2, space="PSUM") as psum_a_pool,
        ):
            # ---- replicated constants ----
            # w1 chunk-transposed: (c128, mip) per c-chunk
            w1t = []
            for k in range(NCHUNK):
                t = small.tile([128, MIP], F32, name=f"w1t{k}", tag=f"w1t{k}")
                nc.scalar.dma_start(t[:], w1_d[:, k * 128:(k + 1) * 128].rearrange("m c -> c m"))
                w1t.append(t)
            # w2/w3 transposed: (mip, C)
            w2t = small.tile([MIP, C], F32, tag="w2t")
            nc.scalar.dma_start(w2t[:], w2_d.rearrange("o m -> m o"))
            w3t = small.tile([MIP, C], F32, tag="w3t")
            nc.scalar.dma_start(w3t[:], w3_d.rearrange("o m -> m o"))
            # b2/b3 per-partition: (128, chunk)
            b2t = small.tile([128, NCHUNK], F32, tag="b2t")
            nc.scalar.dma_start(b2t[:], b2_d.rearrange("(k p) -> p k", p=128))
            b3t = small.tile([128, NCHUNK], F32, tag="b3t")
            nc.scalar.dma_start(b3t[:], b3_d.rearrange("(k p) -> p k", p=128))
            # BN constants, (mip, 1) per-partition scalars
            bn_in = small.tile([MIP, 5], F32, tag="bn_in")
            for i, d in enumerate((var_d, gamma_d, beta_d, mean_d, b1_d)):
                nc.scalar.dma_start(bn_in[:, i:i + 1], d[:].unsqueeze(1))
            var_c = bn_in[:, 0:1]
            gamma_c = bn_in[:, 1:2]
            beta_c = bn_in[:, 2:3]
            mean_c = bn_in[:, 3:4]
            b1_c = bn_in[:, 4:5]

            consts = small.tile([128, 2], F32, tag="consts")
            nc.vector.memset(consts[:, 0:1], EPS)
            nc.vector.memset(consts[:, 1:2], 3.0)

            bn_t = small.tile([MIP, 4], F32, tag="bn_t")
            sv = bn_t[:, 0:1]       # sqrt(var+eps)
            inv = bn_t[:, 1:2]      # gamma / sqrt(var+eps)
            scale_p = bn_t[:, 2:3]  # inv / W   (pool-sum -> mean fold)
            bias_p = bn_t[:, 3:4]   # (b1 - mean) * inv + beta
            nc.scalar.activation(sv, var_c, Act.Sqrt, bias=consts[:MIP, 0:1], scale=1.0)
            nc.vector.reciprocal(inv, sv)
            nc.vector.tensor_tensor(inv, inv, gamma_c, Alu.mult)
            nc.vector.tensor_scalar_mul(scale_p, inv, 1.0 / W)
            nc.vector.tensor_tensor(bias_p, b1_c, mean_c, Alu.subtract)
            nc.vector.scalar_tensor_tensor(bias_p, bias_p, inv, beta_c, Alu.mult, Alu.add)

            xt = {}
            rt = {}
            ah2 = {}
            aw = {}

            # ---- phase 1 (per batch): load, conv, pools, attention ----
            for b in range(NLOC):
                for j in range(NHALF):
                    js = slice(j * HCOL, (j + 1) * HCOL)
                    for k in range(NCHUNK):
                        cs = slice(k * 128, (k + 1) * 128)
                        t = big.tile([128, HCOL], F32, name=f"x_{b}_{k}_{j}", tag=f"x{b}{k}{j}")
                        nc.sync.dma_start(t[:], x_d[b, cs].rearrange("c h w -> c (h w)")[:, js])
                        xt[b, k, j] = t
                        t = big.tile([128, HCOL], F32, name=f"r_{b}_{k}_{j}", tag=f"r{b}{k}{j}")
                        nc.sync.dma_start(t[:], r_d[b, cs].rearrange("c h w -> c (h w)")[:, js])
                        rt[b, k, j] = t

                # conv (c -> mip) + implicit x+res via PSUM accumulation,
                # then directional pool sums
                hp = tc.high_priority()
                hp.__enter__()
                yh_sum = work.tile([MIP, H], F32, name=f"yh_{b}", tag="yh")
                ywp = work.tile([MIP, SEG * W], F32, name=f"ywp_{b}", tag="ywp")
                if "conv" not in stages or "pools" not in stages:
                    nc.vector.memset(yh_sum[:], 0.01)
                    nc.vector.memset(ywp[:], 0.01)
                for s in range(SEG if "conv" in stages else 0):
                    # segment s covers h rows [s*SEGH, (s+1)*SEGH) = half tile s
                    ypsum = psum_y_pool.tile([MIP, SEGCOL], F32, name=f"yp_{b}_{s}", tag="yp")
                    for jj in range(0, SEGCOL, 512):
                        srcs = [(k, t) for k in range(NCHUNK)
                                for t in (xt[b, k, s], rt[b, k, s])]
                        for i, (k, src) in enumerate(srcs):
                            nc.tensor.matmul(
                                ypsum[:, jj:jj + 512],
                                w1t[k][:, :MIP].bitcast(F32R),
                                src[:, jj:jj + 512].bitcast(F32R),
                                start=(i == 0),
                                stop=(i == len(srcs) - 1),
                            )
                    if "pools" in stages:
                        # row sums (over w) for this segment's h rows
                        nc.vector.reduce_sum(
                            yh_sum[:, s * SEGH:(s + 1) * SEGH],
                            ypsum.rearrange("m (h w) -> m h w", h=SEGH),
                            axis=AX.X,
                        )
                        # partial column sums (over this segment's h rows)
                        nc.vector.reduce_sum(
                            ywp[:, s * W:(s + 1) * W],
                            ypsum.rearrange("m (h w) -> m w h", h=SEGH),
                            axis=AX.X,
                        )
                yw_sum = work.tile([MIP, W], F32, name=f"yw_{b}", tag="yw")
                nc.vector.tensor_tensor(yw_sum[:], ywp[:, 0:W], ywp[:, W:2 * W], Alu.add)

                # BN (folded) + hswish on the two tiny pooled maps
                if "mlp" in stages:
                    vhw = work.tile([MIP, H + W], F32, name=f"vhw_{b}", tag="vhw")
                    uhw = work.tile([MIP, H + W], F32, name=f"uhw_{b}", tag="uhw")
                    ybn = work.tile([MIP, H + W], F32, name=f"ybn_{b}", tag="ybn")
                    nc.scalar.activation(ybn[:, :H], yh_sum[:], Act.Identity, bias=bias_p, scale=scale_p)
                    nc.scalar.activation(ybn[:, H:], yw_sum[:], Act.Identity, bias=bias_p, scale=scale_p)
                    # hswish(y) = y * min(relu(y+3), 6) / 6
                    nc.scalar.activation(uhw[:], ybn[:], Act.Relu, bias=consts[:MIP, 1:2], scale=1.0)
                    nc.vector.tensor_scalar_min(uhw[:], uhw[:], 6.0)
                    nc.vector.scalar_tensor_tensor(vhw[:], uhw[:], 1.0 / 6.0, ybn[:], Alu.mult, Alu.mult)
                    v_h = vhw[:, :H]
                    v_w = vhw[:, H:]

                # attention vectors: a_h2 = 2*sigmoid(w2 @ v_h + b2),
                # a_w = sigmoid(w3 @ v_w + b3)
                for k in range(NCHUNK):
                    cs = slice(k * 128, (k + 1) * 128)
                    aht = work.tile([128, H], F32, name=f"ah_{b}_{k}", tag=f"ah{k}")
                    awt = work.tile([128, W], F32, name=f"aw_{b}_{k}", tag=f"aw{k}")
                    if "mlp" in stages:
                        ahp = psum_a_pool.tile([128, H], F32, name=f"ahp_{b}_{k}", tag="ahp")
                        nc.tensor.matmul(ahp[:], w2t[:, cs], v_h, start=True, stop=True)
                        # fold the final x2 into a_h via activation scale on copy
                        nc.scalar.activation(aht[:], ahp[:], Act.Sigmoid, bias=b2t[:, k:k + 1], scale=1.0)
                        nc.scalar.mul(aht[:], aht[:], 2.0)
                        awp = psum_a_pool.tile([128, W], F32, name=f"awp_{b}_{k}", tag="awp")
                        nc.tensor.matmul(awp[:], w3t[:, cs], v_w, start=True, stop=True)
                        nc.scalar.activation(awt[:], awp[:], Act.Sigmoid, bias=b3t[:, k:k + 1], scale=1.0)
                    else:
                        nc.vector.memset(aht[:], 1.0)
                        nc.vector.memset(awt[:], 0.5)
                    ah2[b, k] = aht
                    aw[b, k] = awt
                hp.__exit__(None, None, None)

            # ---- phase 2 (per batch): final elementwise + store ----
            # out = (2*a_h*a_w)*x + 2*res
            for b in range(NLOC):
                for k in range(NCHUNK):
                    cs = slice(k * 128, (k + 1) * 128)
                    od = out_d[b, cs].rearrange("c h w -> c (h w)")
                    for j in range(NHALF):
                        h0 = j * (H // NHALF)
                        h1 = (j + 1) * (H // NHALF)
                        nh = h1 - h0
                        eng = "gp" if (k, j) in GP_UNITS else "dve"
                        if f"final_{eng}" not in stages:
                            continue
                        xs_ = xt[b, k, j].rearrange("p (h w) -> p h w", h=nh)
                        rs_ = rt[b, k, j].rearrange("p (h w) -> p h w", h=nh)
                        awb = aw[b, k].unsqueeze(1).broadcast_to((128, nh, W))
                        ahb = ah2[b, k][:, h0:h1].unsqueeze(2).broadcast_to((128, nh, W))
                        if eng == "dve":
                            nc.vector.tensor_tensor(xs_, xs_, awb, Alu.mult)
                            nc.vector.tensor_tensor(xs_, xs_, ahb, Alu.mult)
                            nc.vector.scalar_tensor_tensor(rs_, rs_, 2.0, xs_, Alu.mult, Alu.add)
                        elif GP_STT:
                            nc.gpsimd.scalar_tensor_tensor(xs_, xs_, 1.0, awb, Alu.mult, Alu.mult)
                            nc.gpsimd.scalar_tensor_tensor(xs_, xs_, 1.0, ahb, Alu.mult, Alu.mult)
                            nc.gpsimd.scalar_tensor_tensor(rs_, rs_, 2.0, xs_, Alu.mult, Alu.add)
                        else:
                            nc.gpsimd.tensor_tensor(xs_, xs_, awb, Alu.mult)
                            nc.gpsimd.tensor_tensor(xs_, xs_, ahb, Alu.mult)
                            nc.gpsimd.tensor_scalar_mul(rs_, rs_, 2.0)
                            nc.gpsimd.tensor_tensor(rs_, rs_, xs_, Alu.add)

                        # result lives in rt[b, k, j]
                        nc.sync.dma_start(od[:, j * HCOL:(j + 1) * HCOL], rt[b, k, j][:])

    nc.compile()
    return nc


_NC_CACHE = None


def _get_module():
    global _NC_CACHE
    if _NC_CACHE is None:
        _NC_CACHE = build_module()
    return _NC_CACHE


def make_in_maps(inputs):
    reps = {k: np.ascontiguousarray(v) for k, v in inputs.items()
            if k not in ("x", "residual")}
    in_maps = []
    for core in range(N_CORES):
        bs = slice(core * NLOC, (core + 1) * NLOC)
        m = {"x": np.ascontiguousarray(inputs["x"][bs]),
             "residual": np.ascontiguousarray(inputs["residual"][bs])}
        m.update(reps)
        in_maps.append(m)
    return in_maps


def run_spmd(nc, in_maps):
    res = run_bass_kernel_spmd(nc, in_maps, core_ids=list(range(N_CORES)))
    return np.concatenate([res.results[c]["out"] for c in range(N_CORES)], axis=0)


def kernel(**inputs):
    inputs = {k: np.asarray(v) for k, v in inputs.items()}
    nc = _get_module()
    return run_spmd(nc, make_in_maps(inputs))


# revision 17
# speedup vs baseline: 1.4686x; 1.1041x over previous
"""CoordAtt Trainium2 Bass kernel.

Reference computation (per batch n, c=256, h=w=64, mip=8):
    xs   = x + residual                      (bilinear resize of residual at
                                              identical shape is the identity)
    y    = concat(mean_w(xs), mean_h(xs))    -> [c, h+w]
    y    = hswish(BN(w1 @ y + b1))           -> [mip, h+w]
    a_h  = sigmoid(w2 @ y[:, :h] + b2)       -> [c, h]
    a_w  = sigmoid(w3 @ y[:, h:] + b3)       -> [c, w]
    out  = 2*xs*a_h*a_w + 2*residual*(1 - a_h*a_w)
         = 2*(a_h*a_w*x + residual)          (algebraically identical)

Kernel strategy (8 cores, data-parallel over batch n: 2 batches/core):
  * conv-before-pool: pooling and the 1x1 conv are both linear, so compute
    y_conv = w1^T @ x + w1^T @ res on the TensorEngine (PSUM accumulation
    gives the x+res add for free), then pool the tiny (mip, h*w) result.
    Matmuls run in float32r mode: fp32 data at full PE rate.
  * BN folds into one per-partition scale/bias activation op.
  * final elementwise tail is only 3 ops/element, split across VectorE (DVE)
    and GpSimd on h-half tiles (separate SBUF tensors per engine --
    concurrent DVE+GpSimd in-place writes to one tensor hang the device),
    all in-place in the input tiles.
  * two emission phases (all pool/attention work for every batch first, then
    all finals) so batch i+1's pipeline overlaps batch i's elementwise tail.
"""

import numpy as np

import concourse.bacc as bacc
import concourse.mybir as mybir
from concourse.tile import TileContext
from concourse.bass_utils import run_bass_kernel_spmd

F32 = mybir.dt.float32
F32R = mybir.dt.float32r
Alu = mybir.AluOpType
Act = mybir.ActivationFunctionType
AX = mybir.AxisListType

N_CORES = 8
N, C, H, W = 16, 256, 64, 64
NLOC = N // N_CORES           # batches per core
MIP = 8
EPS = 1e-5
HW = H * W                    # 4096 free columns per (batch, c-chunk)
NCHUNK = C // 128             # c-chunk count (2)
NHALF = 2                     # h-half split of each chunk tile
HCOL = HW // NHALF            # 2048 columns per half tile
SEG = 2                       # conv psum segments per batch: 2 x 2048 cols
SEGH = H // SEG               # h rows per segment (32)
SEGCOL = SEGH * W             # columns per segment (2048)

# final elementwise: which (chunk, half) units go to GpSimd (rest on DVE)
GP_UNITS = frozenset({(0, 1), (1, 1)})
GP_STT = True                 # use fused scalar_tensor_tensor on GpSimd

ALL_STAGES = frozenset({"conv", "pools", "mlp", "final_dve", "final_gp"})


def build_module(stages=ALL_STAGES):
    nc = bacc.Bacc("TRN2", target_bir_lowering=False)

    x_d = nc.dram_tensor("x", (NLOC, C, H, W), F32, kind="ExternalInput")
    r_d = nc.dram_tensor("residual", (NLOC, C, H, W), F32, kind="ExternalInput")
    w1_d = nc.dram_tensor("w1", (MIP, C), F32, kind="ExternalInput")
    b1_d = nc.dram_tensor("b1", (MIP,), F32, kind="ExternalInput")
    gamma_d = nc.dram_tensor("bn_gamma", (MIP,), F32, kind="ExternalInput")
    beta_d = nc.dram_tensor("bn_beta", (MIP,), F32, kind="ExternalInput")
    mean_d = nc.dram_tensor("bn_mean", (MIP,), F32, kind="ExternalInput")
    var_d = nc.dram_tensor("bn_var", (MIP,), F32, kind="ExternalInput")
    w2_d = nc.dram_tensor("w2", (C, MIP), F32, kind="ExternalInput")
    b2_d = nc.dram_tensor("b2", (C,), F32, kind="ExternalInput")
    w3_d = nc.dram_tensor("w3", (C, MIP), F32, kind="ExternalInput")
    b3_d = nc.dram_tensor("b3", (C,), F32, kind="ExternalInput")
    out_d = nc.dram_tensor("out", (NLOC, C, H, W), F32, kind="ExternalOutput")

    with TileContext(nc) as tc:
        with (
            tc.tile_pool(name="big", bufs=1) as big,
            tc.tile_pool(name="small", bufs=1) as small,
            tc.tile_pool(name="work", bufs=2) as work,
            tc.tile_pool(name="psum_y", bufs=1, space="PSUM") as psum_y_pool,
            tc.tile_pool(name="psum_a", bufs=2, space="PSUM") as psum_a_pool,
        ):
            # ---- replicated constants ----
            # w1 chunk-transposed: (c128, mip) per c-chunk
            w1t = []
            for k in range(NCHUNK):
                t = small.tile([128, MIP], F32, name=f"w1t{k}", tag=f"w1t{k}")
                nc.scalar.dma_start(t[:], w1_d[:, k * 128:(k + 1) * 128].rearrange("m c -> c m"))
                w1t.append(t)
            # w2/w3 transposed: (mip, C)
            w2t = small.tile([MIP, C], F32, tag="w2t")
            nc.scalar.dma_start(w2t[:], w2_d.rearrange("o m -> m o"))
            w3t = small.tile([MIP, C], F32, tag="w3t")
            nc.scalar.dma_start(w3t[:], w3_d.rearrange("o m -> m o"))
            # b2/b3 per-partition: (128, chunk)
            b2t = small.tile([128, NCHUNK], F32, tag="b2t")
            nc.scalar.dma_start(b2t[:], b2_d.rearrange("(k p) -> p k", p=128))
            b3t = small.tile([128, NCHUNK], F32, tag="b3t")
            nc.scalar.dma_start(b3t[:], b3_d.rearrange("(k p) -> p k", p=128))
            # BN constants, (mip, 1) per-partition scalars
            bn_in = small.tile([MIP, 5], F32, tag="bn_in")
            for i, d in enumerate((var_d, gamma_d, beta_d, mean_d, b1_d)):
                nc.scalar.dma_start(bn_in[:, i:i + 1], d[:].unsqueeze(1))
            var_c = bn_in[:, 0:1]
            gamma_c = bn_in[:, 1:2]
            beta_c = bn_in[:, 2:3]
            mean_c = bn_in[:, 3:4]
            b1_c = bn_in[:, 4:5]

            consts = small.tile([128, 2], F32, tag="consts")
            nc.vector.memset(consts[:, 0:1], EPS)
            nc.vector.memset(consts[:, 1:2], 3.0)

            bn_t = small.tile([MIP, 4], F32, tag="bn_t")
            sv = bn_t[:, 0:1]       # sqrt(var+eps)
            inv = bn_t[:, 1:2]      # gamma / sqrt(var+eps)
            scale_p = bn_t[:, 2:3]  # inv / W   (pool-sum -> mean fold)
            bias_p = bn_t[:, 3:4]   # (b1 - mean) * inv + beta
            nc.scalar.activation(sv, var_c, Act.Sqrt, bias=consts[:MIP, 0:1], scale=1.0)
            nc.vector.reciprocal(inv, sv)
            nc.vector.tensor_tensor(inv, inv, gamma_c, Alu.mult)
            nc.vector.tensor_scalar_mul(scale_p, inv, 1.0 / W)
            nc.vector.tensor_tensor(bias_p, b1_c, mean_c, Alu.subtract)
            nc.vector.scalar_tensor_tensor(bias_p, bias_p, inv, beta_c, Alu.mult, Alu.add)

            xt = {}
            rt = {}
            ah2 = {}
            aw = {}

            # ---- phase 1 (per batch): load, conv, pools, attention ----
            for b in range(NLOC):
                for j in range(NHALF):
                    js = slice(j * HCOL, (j + 1) * HCOL)
                    for k in range(NCHUNK):
                        cs = slice(k * 128, (k + 1) * 128)
                        t = big.tile([128, HCOL], F32, name=f"x_{b}_{k}_{j}", tag=f"x{b}{k}{j}")
                        nc.sync.dma_start(t[:], x_d[b, cs].rearrange("c h w -> c (h w)")[:, js])
                        xt[b, k, j] = t
                        t = big.tile([128, HCOL], F32, name=f"r_{b}_{k}_{j}", tag=f"r{b}{k}{j}")
                        nc.sync.dma_start(t[:], r_d[b, cs].rearrange("c h w -> c (h w)")[:, js])
                        rt[b, k, j] = t

                # conv (c -> mip) + implicit x+res via PSUM accumulation,
                # then directional pool sums.  The a_h attention path only
                # needs row pools of its own h-segment, so it is computed per
                # segment and the a_h multiply of the finals starts before the
                # whole batch has even arrived.
                assert "conv" in stages and "pools" in stages and "mlp" in stages
                yh_sum = work.tile([MIP, H], F32, name=f"yh_{b}", tag="yh")
                ywp = work.tile([MIP, SEG * W], F32, name=f"ywp_{b}", tag="ywp")
                aht_k = []
                for k in range(NCHUNK):
                    aht = work.tile([128, H], F32, name=f"ah_{b}_{k}", tag=f"ah{k}")
                    aht_k.append(aht)
                    ah2[b, k] = aht
                for s in range(SEG):
                    # segment s covers h rows [s*SEGH, (s+1)*SEGH) = half tile s
                    hs = slice(s * SEGH, (s + 1) * SEGH)
                    with tc.high_priority():
                        ypsum = psum_y_pool.tile([MIP, SEGCOL], F32, name=f"yp_{b}_{s}", tag="yp")
                        for jj in range(0, SEGCOL, 512):
                            srcs = [(k, t) for k in range(NCHUNK)
                                    for t in (xt[b, k, s], rt[b, k, s])]
                            for i, (k, src) in enumerate(srcs):
                                nc.tensor.matmul(
                                    ypsum[:, jj:jj + 512],
                                    w1t[k][:, :MIP].bitcast(F32R),
                                    src[:, jj:jj + 512].bitcast(F32R),
                                    start=(i == 0),
                                    stop=(i == len(srcs) - 1),
                                )
                        # row sums (over w) for this segment's h rows
                        nc.vector.reduce_sum(
                            yh_sum[:, hs],
                            ypsum.rearrange("m (h w) -> m h w", h=SEGH),
                            axis=AX.X,
                        )
                        # partial column sums (over this segment's h rows)
                        nc.vector.reduce_sum(
                            ywp[:, s * W:(s + 1) * W],
                            ypsum.rearrange("m (h w) -> m w h", h=SEGH),
                            axis=AX.X,
                        )
                        # staggered a_h path for this segment's rows:
                        # BN + hswish + 1x1 conv + sigmoid (x2 folded in)
                        ybn_s = work.tile([MIP, SEGH], F32, name=f"ybnh_{b}_{s}", tag="ybnh", bufs=4)
                        u_s = work.tile([MIP, SEGH], F32, name=f"uh_{b}_{s}", tag="uh", bufs=4)
                        v_s = work.tile([MIP, SEGH], F32, name=f"vh_{b}_{s}", tag="vh", bufs=4)
                        nc.scalar.activation(ybn_s[:], yh_sum[:, hs], Act.Identity, bias=bias_p, scale=scale_p)
                        nc.scalar.activation(u_s[:], ybn_s[:], Act.Relu, bias=consts[:MIP, 1:2], scale=1.0)
                        nc.vector.tensor_scalar_min(u_s[:], u_s[:], 6.0)
                        nc.vector.scalar_tensor_tensor(v_s[:], u_s[:], 1.0 / 6.0, ybn_s[:], Alu.mult, Alu.mult)
                        for k in range(NCHUNK):
                            cs = slice(k * 128, (k + 1) * 128)
                            ahp = psum_a_pool.tile([128, SEGH], F32, name=f"ahp_{b}_{s}_{k}", tag="ahp")
                            nc.tensor.matmul(ahp[:], w2t[:, cs], v_s[:], start=True, stop=True)
                            nc.scalar.activation(aht_k[k][:, hs], ahp[:], Act.Sigmoid, bias=b2t[:, k:k + 1], scale=1.0)
                            nc.scalar.mul(aht_k[k][:, hs], aht_k[k][:, hs], 2.0)
                    # staggered first final multiply: x *= a_h2 on half tile s
                    for k in range(NCHUNK):
                        if f"final_{'gp' if (k, s) in GP_UNITS else 'dve'}" not in stages:
                            continue
                        xs_ = xt[b, k, s].rearrange("p (h w) -> p h w", h=SEGH)
                        ahb = aht_k[k][:, hs].unsqueeze(2).broadcast_to((128, SEGH, W))
                        if (k, s) in GP_UNITS:
                            nc.gpsimd.scalar_tensor_tensor(xs_, xs_, 1.0, ahb, Alu.mult, Alu.mult)
                        else:
                            nc.vector.tensor_tensor(xs_, xs_, ahb, Alu.mult)

                # a_w path needs column pools over all h: finish it now
                with tc.high_priority():
                    yw_sum = work.tile([MIP, W], F32, name=f"yw_{b}", tag="yw")
                    nc.vector.tensor_tensor(yw_sum[:], ywp[:, 0:W], ywp[:, W:2 * W], Alu.add)
                    ybn_w = work.tile([MIP, W], F32, name=f"ybnw_{b}", tag="ybnw")
                    u_w = work.tile([MIP, W], F32, name=f"uw_{b}", tag="uw")
                    v_w = work.tile([MIP, W], F32, name=f"vw_{b}", tag="vw")
                    nc.scalar.activation(ybn_w[:], yw_sum[:], Act.Identity, bias=bias_p, scale=scale_p)
                    nc.scalar.activation(u_w[:], ybn_w[:], Act.Relu, bias=consts[:MIP, 1:2], scale=1.0)
                    nc.vector.tensor_scalar_min(u_w[:], u_w[:], 6.0)
                    nc.vector.scalar_tensor_tensor(v_w[:], u_w[:], 1.0 / 6.0, ybn_w[:], Alu.mult, Alu.mult)
                    for k in range(NCHUNK):
                        cs = slice(k * 128, (k + 1) * 128)
                        awt = work.tile([128, W], F32, name=f"aw_{b}_{k}", tag=f"aw{k}")
                        awp = psum_a_pool.tile([128, W], F32, name=f"awp_{b}_{k}", tag="awp")
                        nc.tensor.matmul(awp[:], w3t[:, cs], v_w[:], start=True, stop=True)
                        nc.scalar.activation(awt[:], awp[:], Act.Sigmoid, bias=b3t[:, k:k + 1], scale=1.0)
                        aw[b, k] = awt

            # ---- phase 2 (per batch): final elementwise + store ----
            # out = (2*a_h*a_w)*x + 2*res
            for b in range(NLOC):
                for k in range(NCHUNK):
                    cs = slice(k * 128, (k + 1) * 128)
                    od = out_d[b, cs].rearrange("c h w -> c (h w)")
                    for j in range(NHALF):
                        h0 = j * (H // NHALF)
                        h1 = (j + 1) * (H // NHALF)
                        nh = h1 - h0
                        eng = "gp" if (k, j) in GP_UNITS else "dve"
                        if f"final_{eng}" not in stages:
                            continue
                        xs_ = xt[b, k, j].rearrange("p (h w) -> p h w", h=nh)
                        rs_ = rt[b, k, j].rearrange("p (h w) -> p h w", h=nh)
                        awb = aw[b, k].unsqueeze(1).broadcast_to((128, nh, W))
                        if eng == "dve":
                            nc.vector.tensor_tensor(xs_, xs_, awb, Alu.mult)
                            nc.vector.scalar_tensor_tensor(rs_, rs_, 2.0, xs_, Alu.mult, Alu.add)
                        else:
                            nc.gpsimd.scalar_tensor_tensor(xs_, xs_, 1.0, awb, Alu.mult, Alu.mult)
                            nc.gpsimd.scalar_tensor_tensor(rs_, rs_, 2.0, xs_, Alu.mult, Alu.add)

                        # result lives in rt[b, k, j]
                        nc.sync.dma_start(od[:, j * HCOL:(j + 1) * HCOL], rt[b, k, j][:])

    nc.compile()
    return nc


_NC_CACHE = None


def _get_module():
    global _NC_CACHE
    if _NC_CACHE is None:
        _NC_CACHE = build_module()
    return _NC_CACHE


def make_in_maps(inputs):
    reps = {k: np.ascontiguousarray(v) for k, v in inputs.items()
            if k not in ("x", "residual")}
    in_maps = []
    for core in range(N_CORES):
        bs = slice(core * NLOC, (core + 1) * NLOC)
        m = {"x": np.ascontiguousarray(inputs["x"][bs]),
             "residual": np.ascontiguousarray(inputs["residual"][bs])}
        m.update(reps)
        in_maps.append(m)
    return in_maps


def run_spmd(nc, in_maps):
    res = run_bass_kernel_spmd(nc, in_maps, core_ids=list(range(N_CORES)))
    return np.concatenate([res.results[c]["out"] for c in range(N_CORES)], axis=0)


def kernel(**inputs):
    inputs = {k: np.asarray(v) for k, v in inputs.items()}
    nc = _get_module()
    return run_spmd(nc, make_in_maps(inputs))


# revision 18
# speedup vs baseline: 1.5262x; 1.0392x over previous
"""CoordAtt Trainium2 Bass kernel.

Reference computation (per batch n, c=256, h=w=64, mip=8):
    xs   = x + residual                      (bilinear resize of residual at
                                              identical shape is the identity)
    y    = concat(mean_w(xs), mean_h(xs))    -> [c, h+w]
    y    = hswish(BN(w1 @ y + b1))           -> [mip, h+w]
    a_h  = sigmoid(w2 @ y[:, :h] + b2)       -> [c, h]
    a_w  = sigmoid(w3 @ y[:, h:] + b3)       -> [c, w]
    out  = 2*xs*a_h*a_w + 2*residual*(1 - a_h*a_w)
         = 2*(a_h*a_w*x + residual)          (algebraically identical)

Kernel strategy (8 cores, data-parallel over batch n: 2 batches/core):
  * conv-before-pool: pooling and the 1x1 conv are both linear, so compute
    y_conv = w1^T @ x + w1^T @ res on the TensorEngine (PSUM accumulation
    gives the x+res add for free), then pool the tiny (mip, h*w) result.
    Matmuls run in float32r mode: fp32 data at full PE rate.
  * BN folds into one per-partition scale/bias activation op.
  * final elementwise tail is only 3 ops/element, split across VectorE (DVE)
    and GpSimd on h-half tiles (separate SBUF tensors per engine --
    concurrent DVE+GpSimd in-place writes to one tensor hang the device),
    all in-place in the input tiles.
  * two emission phases (all pool/attention work for every batch first, then
    all finals) so batch i+1's pipeline overlaps batch i's elementwise tail.
"""

import numpy as np

import concourse.bacc as bacc
import concourse.mybir as mybir
from concourse.tile import TileContext
from concourse.bass_utils import run_bass_kernel_spmd

F32 = mybir.dt.float32
F32R = mybir.dt.float32r
Alu = mybir.AluOpType
Act = mybir.ActivationFunctionType
AX = mybir.AxisListType

N_CORES = 8
N, C, H, W = 16, 256, 64, 64
NLOC = N // N_CORES           # batches per core
MIP = 8
EPS = 1e-5
HW = H * W                    # 4096 free columns per (batch, c-chunk)
NCHUNK = C // 128             # c-chunk count (2)
NHALF = 2                     # h-half split of each chunk tile
HCOL = HW // NHALF            # 2048 columns per half tile
SEG = 2                       # conv psum segments per batch: 2 x 2048 cols
SEGH = H // SEG               # h rows per segment (32)
SEGCOL = SEGH * W             # columns per segment (2048)

# final elementwise: which (chunk, half) units go to GpSimd (rest on DVE),
# per batch: give GpSimd the late halves of batch 0 but the early halves of
# batch 1, so the tail of the last batch lands on the faster DVE
GP_UNITS_BY_BATCH = (frozenset({(0, 1), (1, 1)}), frozenset({(0, 0), (1, 0)}))
GP_STT = True                 # use fused scalar_tensor_tensor on GpSimd

ALL_STAGES = frozenset({"conv", "pools", "mlp", "final_dve", "final_gp"})


def build_module(stages=ALL_STAGES):
    nc = bacc.Bacc("TRN2", target_bir_lowering=False)

    x_d = nc.dram_tensor("x", (NLOC, C, H, W), F32, kind="ExternalInput")
    r_d = nc.dram_tensor("residual", (NLOC, C, H, W), F32, kind="ExternalInput")
    w1_d = nc.dram_tensor("w1", (MIP, C), F32, kind="ExternalInput")
    b1_d = nc.dram_tensor("b1", (MIP,), F32, kind="ExternalInput")
    gamma_d = nc.dram_tensor("bn_gamma", (MIP,), F32, kind="ExternalInput")
    beta_d = nc.dram_tensor("bn_beta", (MIP,), F32, kind="ExternalInput")
    mean_d = nc.dram_tensor("bn_mean", (MIP,), F32, kind="ExternalInput")
    var_d = nc.dram_tensor("bn_var", (MIP,), F32, kind="ExternalInput")
    w2_d = nc.dram_tensor("w2", (C, MIP), F32, kind="ExternalInput")
    b2_d = nc.dram_tensor("b2", (C,), F32, kind="ExternalInput")
    w3_d = nc.dram_tensor("w3", (C, MIP), F32, kind="ExternalInput")
    b3_d = nc.dram_tensor("b3", (C,), F32, kind="ExternalInput")
    out_d = nc.dram_tensor("out", (NLOC, C, H, W), F32, kind="ExternalOutput")

    with TileContext(nc) as tc:
        with (
            tc.tile_pool(name="big", bufs=1) as big,
            tc.tile_pool(name="small", bufs=1) as small,
            tc.tile_pool(name="work", bufs=2) as work,
            tc.tile_pool(name="psum_y", bufs=1, space="PSUM") as psum_y_pool,
            tc.tile_pool(name="psum_a", bufs=2, space="PSUM") as psum_a_pool,
        ):
            # ---- replicated constants ----
            # w1 chunk-transposed: (c128, mip) per c-chunk
            w1t = []
            for k in range(NCHUNK):
                t = small.tile([128, MIP], F32, name=f"w1t{k}", tag=f"w1t{k}")
                nc.scalar.dma_start(t[:], w1_d[:, k * 128:(k + 1) * 128].rearrange("m c -> c m"))
                w1t.append(t)
            # w2/w3 transposed: (mip, C)
            w2t = small.tile([MIP, C], F32, tag="w2t")
            nc.scalar.dma_start(w2t[:], w2_d.rearrange("o m -> m o"))
            w3t = small.tile([MIP, C], F32, tag="w3t")
            nc.scalar.dma_start(w3t[:], w3_d.rearrange("o m -> m o"))
            # b2/b3 per-partition: (128, chunk)
            b2t = small.tile([128, NCHUNK], F32, tag="b2t")
            nc.scalar.dma_start(b2t[:], b2_d.rearrange("(k p) -> p k", p=128))
            b3t = small.tile([128, NCHUNK], F32, tag="b3t")
            nc.scalar.dma_start(b3t[:], b3_d.rearrange("(k p) -> p k", p=128))
            # BN constants, (mip, 1) per-partition scalars
            bn_in = small.tile([MIP, 5], F32, tag="bn_in")
            for i, d in enumerate((var_d, gamma_d, beta_d, mean_d, b1_d)):
                nc.scalar.dma_start(bn_in[:, i:i + 1], d[:].unsqueeze(1))
            var_c = bn_in[:, 0:1]
            gamma_c = bn_in[:, 1:2]
            beta_c = bn_in[:, 2:3]
            mean_c = bn_in[:, 3:4]
            b1_c = bn_in[:, 4:5]

            consts = small.tile([128, 2], F32, tag="consts")
            nc.vector.memset(consts[:, 0:1], EPS)
            nc.vector.memset(consts[:, 1:2], 3.0)

            bn_t = small.tile([MIP, 4], F32, tag="bn_t")
            sv = bn_t[:, 0:1]       # sqrt(var+eps)
            inv = bn_t[:, 1:2]      # gamma / sqrt(var+eps)
            scale_p = bn_t[:, 2:3]  # inv / W   (pool-sum -> mean fold)
            bias_p = bn_t[:, 3:4]   # (b1 - mean) * inv + beta
            nc.scalar.activation(sv, var_c, Act.Sqrt, bias=consts[:MIP, 0:1], scale=1.0)
            nc.vector.reciprocal(inv, sv)
            nc.vector.tensor_tensor(inv, inv, gamma_c, Alu.mult)
            nc.vector.tensor_scalar_mul(scale_p, inv, 1.0 / W)
            nc.vector.tensor_tensor(bias_p, b1_c, mean_c, Alu.subtract)
            nc.vector.scalar_tensor_tensor(bias_p, bias_p, inv, beta_c, Alu.mult, Alu.add)

            xt = {}
            rt = {}
            ah2 = {}
            aw = {}

            # ---- phase 1 (per batch): load, conv, pools, attention ----
            for b in range(NLOC):
                for j in range(NHALF):
                    js = slice(j * HCOL, (j + 1) * HCOL)
                    for k in range(NCHUNK):
                        cs = slice(k * 128, (k + 1) * 128)
                        t = big.tile([128, HCOL], F32, name=f"x_{b}_{k}_{j}", tag=f"x{b}{k}{j}")
                        nc.sync.dma_start(t[:], x_d[b, cs].rearrange("c h w -> c (h w)")[:, js])
                        xt[b, k, j] = t
                        t = big.tile([128, HCOL], F32, name=f"r_{b}_{k}_{j}", tag=f"r{b}{k}{j}")
                        nc.sync.dma_start(t[:], r_d[b, cs].rearrange("c h w -> c (h w)")[:, js])
                        rt[b, k, j] = t

                # conv (c -> mip) + implicit x+res via PSUM accumulation,
                # then directional pool sums.  The a_h attention path only
                # needs row pools of its own h-segment, so it is computed per
                # segment and the a_h multiply of the finals starts before the
                # whole batch has even arrived.
                assert "conv" in stages and "pools" in stages and "mlp" in stages
                yh_sum = work.tile([MIP, H], F32, name=f"yh_{b}", tag="yh")
                ywp = work.tile([MIP, SEG * W], F32, name=f"ywp_{b}", tag="ywp")
                aht_k = []
                for k in range(NCHUNK):
                    aht = work.tile([128, H], F32, name=f"ah_{b}_{k}", tag=f"ah{k}")
                    aht_k.append(aht)
                    ah2[b, k] = aht
                for s in range(SEG):
                    # segment s covers h rows [s*SEGH, (s+1)*SEGH) = half tile s
                    hs = slice(s * SEGH, (s + 1) * SEGH)
                    with tc.high_priority():
                        ypsum = psum_y_pool.tile([MIP, SEGCOL], F32, name=f"yp_{b}_{s}", tag="yp")
                        for jj in range(0, SEGCOL, 512):
                            srcs = [(k, t) for k in range(NCHUNK)
                                    for t in (xt[b, k, s], rt[b, k, s])]
                            for i, (k, src) in enumerate(srcs):
                                nc.tensor.matmul(
                                    ypsum[:, jj:jj + 512],
                                    w1t[k][:, :MIP].bitcast(F32R),
                                    src[:, jj:jj + 512].bitcast(F32R),
                                    start=(i == 0),
                                    stop=(i == len(srcs) - 1),
                                )
                        # partial column sums first: they gate a_w, which
                        # is the critical path of the whole batch
                        nc.vector.reduce_sum(
                            ywp[:, s * W:(s + 1) * W],
                            ypsum.rearrange("m (h w) -> m w h", h=SEGH),
                            axis=AX.X,
                        )
                        # row sums (over w) for this segment's h rows
                        nc.vector.reduce_sum(
                            yh_sum[:, hs],
                            ypsum.rearrange("m (h w) -> m h w", h=SEGH),
                            axis=AX.X,
                        )
                        # staggered a_h path for this segment's rows:
                        # BN + hswish + 1x1 conv + sigmoid (x2 folded in)
                        ybn_s = work.tile([MIP, SEGH], F32, name=f"ybnh_{b}_{s}", tag="ybnh", bufs=4)
                        u_s = work.tile([MIP, SEGH], F32, name=f"uh_{b}_{s}", tag="uh", bufs=4)
                        v_s = work.tile([MIP, SEGH], F32, name=f"vh_{b}_{s}", tag="vh", bufs=4)
                        nc.scalar.activation(ybn_s[:], yh_sum[:, hs], Act.Identity, bias=bias_p, scale=scale_p)
                        nc.scalar.activation(u_s[:], ybn_s[:], Act.Relu, bias=consts[:MIP, 1:2], scale=1.0)
                        nc.vector.tensor_scalar_min(u_s[:], u_s[:], 6.0)
                        nc.vector.scalar_tensor_tensor(v_s[:], u_s[:], 1.0 / 6.0, ybn_s[:], Alu.mult, Alu.mult)
                        for k in range(NCHUNK):
                            cs = slice(k * 128, (k + 1) * 128)
                            ahp = psum_a_pool.tile([128, SEGH], F32, name=f"ahp_{b}_{s}_{k}", tag="ahp")
                            nc.tensor.matmul(ahp[:], w2t[:, cs], v_s[:], start=True, stop=True)
                            nc.scalar.activation(aht_k[k][:, hs], ahp[:], Act.Sigmoid, bias=b2t[:, k:k + 1], scale=1.0)
                            nc.scalar.mul(aht_k[k][:, hs], aht_k[k][:, hs], 2.0)
                    # staggered first final multiply: x *= a_h2 on half tile s
                    for k in range(NCHUNK):
                        if f"final_{'gp' if (k, s) in GP_UNITS_BY_BATCH[b] else 'dve'}" not in stages:
                            continue
                        xs_ = xt[b, k, s].rearrange("p (h w) -> p h w", h=SEGH)
                        ahb = aht_k[k][:, hs].unsqueeze(2).broadcast_to((128, SEGH, W))
                        if (k, s) in GP_UNITS_BY_BATCH[b]:
                            nc.gpsimd.scalar_tensor_tensor(xs_, xs_, 1.0, ahb, Alu.mult, Alu.mult)
                        else:
                            nc.vector.tensor_tensor(xs_, xs_, ahb, Alu.mult)

                # a_w path needs column pools over all h: finish it now
                with tc.high_priority():
                    yw_sum = work.tile([MIP, W], F32, name=f"yw_{b}", tag="yw")
                    nc.vector.tensor_tensor(yw_sum[:], ywp[:, 0:W], ywp[:, W:2 * W], Alu.add)
                    ybn_w = work.tile([MIP, W], F32, name=f"ybnw_{b}", tag="ybnw")
                    u_w = work.tile([MIP, W], F32, name=f"uw_{b}", tag="uw")
                    v_w = work.tile([MIP, W], F32, name=f"vw_{b}", tag="vw")
                    nc.scalar.activation(ybn_w[:], yw_sum[:], Act.Identity, bias=bias_p, scale=scale_p)
                    nc.scalar.activation(u_w[:], ybn_w[:], Act.Relu, bias=consts[:MIP, 1:2], scale=1.0)
                    nc.vector.tensor_scalar_min(u_w[:], u_w[:], 6.0)
                    nc.vector.scalar_tensor_tensor(v_w[:], u_w[:], 1.0 / 6.0, ybn_w[:], Alu.mult, Alu.mult)
                    for k in range(NCHUNK):
                        cs = slice(k * 128, (k + 1) * 128)
                        awt = work.tile([128, W], F32, name=f"aw_{b}_{k}", tag=f"aw{k}")
                        awp = psum_a_pool.tile([128, W], F32, name=f"awp_{b}_{k}", tag="awp")
                        nc.tensor.matmul(awp[:], w3t[:, cs], v_w[:], start=True, stop=True)
                        nc.scalar.activation(awt[:], awp[:], Act.Sigmoid, bias=b3t[:, k:k + 1], scale=1.0)
                        aw[b, k] = awt

            # ---- phase 2 (per batch): final elementwise + store ----
            # out = (2*a_h*a_w)*x + 2*res
            for b in range(NLOC):
                for k in range(NCHUNK):
                    cs = slice(k * 128, (k + 1) * 128)
                    od = out_d[b, cs].rearrange("c h w -> c (h w)")
                    for j in range(NHALF):
                        h0 = j * (H // NHALF)
                        h1 = (j + 1) * (H // NHALF)
                        nh = h1 - h0
                        eng = "gp" if (k, j) in GP_UNITS_BY_BATCH[b] else "dve"
                        if f"final_{eng}" not in stages:
                            continue
                        xs_ = xt[b, k, j].rearrange("p (h w) -> p h w", h=nh)
                        rs_ = rt[b, k, j].rearrange("p (h w) -> p h w", h=nh)
                        awb = aw[b, k].unsqueeze(1).broadcast_to((128, nh, W))
                        if eng == "dve":
                            nc.vector.tensor_tensor(xs_, xs_, awb, Alu.mult)
                            nc.vector.scalar_tensor_tensor(rs_, rs_, 2.0, xs_, Alu.mult, Alu.add)
                        else:
                            nc.gpsimd.scalar_tensor_tensor(xs_, xs_, 1.0, awb, Alu.mult, Alu.mult)
                            nc.gpsimd.scalar_tensor_tensor(rs_, rs_, 2.0, xs_, Alu.mult, Alu.add)

                        # result lives in rt[b, k, j]
                        nc.sync.dma_start(od[:, j * HCOL:(j + 1) * HCOL], rt[b, k, j][:])

    nc.compile()
    return nc


_NC_CACHE = None


def _get_module():
    global _NC_CACHE
    if _NC_CACHE is None:
        _NC_CACHE = build_module()
    return _NC_CACHE


def make_in_maps(inputs):
    reps = {k: np.ascontiguousarray(v) for k, v in inputs.items()
            if k not in ("x", "residual")}
    in_maps = []
    for core in range(N_CORES):
        bs = slice(core * NLOC, (core + 1) * NLOC)
        m = {"x": np.ascontiguousarray(inputs["x"][bs]),
             "residual": np.ascontiguousarray(inputs["residual"][bs])}
        m.update(reps)
        in_maps.append(m)
    return in_maps


def run_spmd(nc, in_maps):
    res = run_bass_kernel_spmd(nc, in_maps, core_ids=list(range(N_CORES)))
    return np.concatenate([res.results[c]["out"] for c in range(N_CORES)], axis=0)


def kernel(**inputs):
    inputs = {k: np.asarray(v) for k, v in inputs.items()}
    nc = _get_module()
    return run_spmd(nc, make_in_maps(inputs))
